# revision 1
# baseline (speedup 1.0000x reference)
"""Causal self-attention (B=4, T=2048, D=1024, H=16) on 8 Trainium2 NeuronCores.

Sharding: data-parallel over batch (4) x tensor-parallel over heads (2 groups
of 8 heads) = 8 cores. Each core computes q/k/v projections for its 8 heads,
head-local attention, and a partial out-projection; the host sums the two
partials per batch element (the out_proj all-reduce).

On-core layout ("dim-major" activations):
  qT, kT  [128, 4, 2048]  - per head-pair g: partition = channel (2 heads x 64),
                            free = token. Produced by W.T @ x.T matmuls.
  vA      [128, 8, 16, 65]- token-major V per (head, token-tile) with a ones
                            column (65th) so P@V also yields softmax denominators.
  S^T     [128, 512] PSUM - keys on partitions, queries on free dim; causal
                            tiles restrict the matmul to the valid column range,
                            a [128,128] triangular additive mask handles the
                            diagonal band, and the key-padding mask is the
                            per-partition bias of the exp activation.
  yT      [128, 4, 2048]  - normalized attention output, dim-major: feeds
                            out-proj as stationary operand.
All matmuls run in float32r (1 cycle/row at N>=256, ~1.5e-4 rel err).
"""
import numpy as np

import concourse.bass as bass
import concourse.bacc as bacc
import concourse.tile as tile
from concourse import mybir
from concourse.bass_utils import run_bass_kernel_spmd

F32 = mybir.dt.float32
F32R = mybir.dt.float32r
EXP = mybir.ActivationFunctionType.Exp
MULT = mybir.AluOpType.mult
ADD = mybir.AluOpType.add

B, T, D = 4, 2048, 1024
H = 16            # total heads
HD = 64           # head dim
HC = 8            # heads per core
NP = 4            # head pairs per core
NEG = -1.0e6      # additive mask value (exp -> 0 after *0.125)

_CACHE = {}


def _build():
    nc = bacc.Bacc("TRN2", target_bir_lowering=False, debug=False,
                   enable_asserts=False)
    dt_in = {}

    def din(name, shape):
        dt_in[name] = nc.dram_tensor(name, shape, F32, kind="ExternalInput").ap()
        return dt_in[name]

    xt = din("xt", [D, T])            # x[b].T
    wq = din("wq", [D, 512])          # Wq_slice.T
    wk = din("wk", [D, 512])
    wv = din("wv", [D, 512])
    wo = din("wo", [512, D])          # Wo[:, cols].T
    bqt = din("bqt", [128, NP])
    bkt = din("bkt", [128, NP])
    bv = din("bv", [512])
    bo = din("bo", [D])
    tri = din("tri", [128, 128])      # causal 0/1 multiplicative band mask
    keyb = din("keyb", [128, 16])     # key-padding additive bias per k-tile
    qmt = din("qmt", [128, 16])       # query mask, token-major
    out = nc.dram_tensor("out", [T, D], F32, kind="ExternalOutput").ap()

    def bcast128(ap):
        return bass.AP(tensor=ap.tensor, offset=ap.offset, ap=[[0, 128]] + ap.ap)

    with tile.TileContext(nc) as tc:
        cpool = tc.alloc_tile_pool(name="const", bufs=1)
        tri_t = cpool.tile([128, 128], F32)
        keyb_t = cpool.tile([128, 16], F32)
        qmt_t = cpool.tile([128, 16], F32)
        bqt_t = cpool.tile([128, NP], F32)
        bkt_t = cpool.tile([128, NP], F32)
        bvrep = cpool.tile([128, 512], F32)
        borep = cpool.tile([128, D], F32)
        vone_f = cpool.tile([128, HC, 16], F32)
        nc.sync.dma_start(out=tri_t, in_=tri)
        nc.sync.dma_start(out=keyb_t, in_=keyb)
        nc.sync.dma_start(out=qmt_t, in_=qmt)
        nc.sync.dma_start(out=bqt_t, in_=bqt)
        nc.sync.dma_start(out=bkt_t, in_=bkt)
        nc.vector.memset(vone_f, 1.0)

        wopool = tc.alloc_tile_pool(name="wop", bufs=1)
        wo_t = wopool.tile([128, 4, D], F32R)

        qkvpool = tc.alloc_tile_pool(name="qkv", bufs=1)
        qT = qkvpool.tile([128, NP, T], F32R, tag="qT")
        kT = qkvpool.tile([128, NP, T], F32R, tag="kT")
        vA = qkvpool.tile([128, HC, 16, HD + 1], F32R, tag="vA")
        nc.vector.tensor_copy(vA[:, :, :, HD], vone_f)

        # ---------------- Phase 1: QKV projections ----------------
        with (
            tc.tile_pool(name="wts", bufs=1) as wpool,
            tc.tile_pool(name="xts", bufs=2) as xpool,
            tc.tile_pool(name="ps1", bufs=6, space="PSUM") as ps1,
            tc.tile_pool(name="ps1v", bufs=2, space="PSUM") as ps1v,
        ):
            wq_t = wpool.tile([128, 8, 512], F32R, tag="w")
            wk_t = wpool.tile([128, 8, 512], F32R, tag="wk")
            wv_t = wpool.tile([128, 8, 512], F32R, tag="wv")

            def load_w(wdram, wt, split=False):
                for kd in range(8):
                    eng = nc.sync if (split and kd % 2) else nc.gpsimd
                    eng.dma_start(out=wt[:, kd, :],
                                  in_=wdram[kd * 128:(kd + 1) * 128, :].bitcast(F32R))

            SLICES = [(0, 256), (256, 256), (512, 256), (768, 256), (1024, 512), (1536, 512)]

            def load_x(si):
                t0, tl = SLICES[si]
                tiles = []
                for kd in range(8):
                    xtile = xpool.tile([128, 512], F32R, tag=f"x{kd}", name=f"x{si}_{kd}",
                                       padded_shape=[128, 512])
                    # split the critical first slice across the sync and the
                    # otherwise-idle scalar-engine DMA queues
                    eng = nc.scalar if (si == 0 and kd % 2) else nc.sync
                    eng.dma_start(out=xtile[:, 0:tl],
                                  in_=xt[kd * 128:(kd + 1) * 128, t0:t0 + tl].bitcast(F32R))
                    tiles.append(xtile)
                return tiles

            xts_next = load_x(0)
            load_w(wq, wq_t)
            load_w(wk, wk_t)
            nc.sync.dma_start(out=bvrep, in_=bcast128(bv))
            load_w(wv, wv_t)
            nc.gpsimd.dma_start(out=wo_t, in_=wo.rearrange("(g p) n -> p g n", p=128).bitcast(F32R))
            nc.sync.dma_start(out=borep, in_=bcast128(bo))

            for si in range(len(SLICES)):
                t0, tl = SLICES[si]
                xts = xts_next
                if si + 1 < len(SLICES):
                    xts_next = load_x(si + 1)
                for wt, bias_t, outT in ((wq_t, bqt_t, qT), (wk_t, bkt_t, kT)):
                    for g in range(NP):
                        ps = ps1.tile([128, 512], F32, tag="ps1")
                        for kd in range(8):
                            nc.tensor.matmul(ps[:, 0:tl], wt[:, kd, g * 128:(g + 1) * 128],
                                             xts[kd][:, 0:tl],
                                             start=(kd == 0), stop=(kd == 7))
                        nc.vector.tensor_scalar_add(outT[:, g, t0:t0 + tl], ps[:, 0:tl],
                                                    bias_t[:, g:g + 1])
                for tt in range(tl // 128):
                    gtt = (t0 // 128) + tt
                    ps = ps1v.tile([128, 512], F32, tag="psv", name=f"psv{gtt}")
                    for kd in range(8):
                        nc.tensor.matmul(ps, xts[kd][:, tt * 128:(tt + 1) * 128], wv_t[:, kd, :],
                                         start=(kd == 0), stop=(kd == 7))
                    nc.vector.tensor_add(vA[:, :, gtt, 0:HD],
                                         ps.rearrange("p (h d) -> p h d", h=HC),
                                         bvrep.rearrange("p (h d) -> p h d", h=HC))

        # ---------------- Phase 2: attention ----------------
        ypool = tc.alloc_tile_pool(name="yT", bufs=1)
        yT = ypool.tile([128, NP, T], F32R)
        def dim0bc(ap, n):
            # insert a stride-0 dim after the partition dim (broadcast)
            return bass.AP(tensor=ap.tensor, offset=ap.offset,
                           ap=[ap.ap[0], [0, n]] + ap.ap[1:])

        with (
            tc.tile_pool(name="ptile", bufs=8) as ppool,
            tc.tile_pool(name="dro", bufs=2) as dpool,
            tc.tile_pool(name="rec", bufs=2) as rpool,
            tc.tile_pool(name="pss", bufs=2, space="PSUM") as pss,
            tc.tile_pool(name="psy", bufs=2, space="PSUM") as psy,
        ):
            prev_tail = None
            for pr in range(NP):
                for qt in range(4):
                    q0 = qt * 512
                    nk = qt * 4 + 4
                    # two-bank tiles: head A in cols [0,512), head B in [512,1024)
                    ys2 = psy.tile([HD + 1, 1024], F32, tag="y", name=f"y{pr}_{qt}")
                    for kt in range(nk):
                        c = kt * 128 - q0
                        mm_lo = 0 if c < 0 else min(c, 256)
                        lo = max(c, 0)
                        s2 = pss.tile([128, 1024], F32, tag="s", name=f"s{pr}_{qt}_{kt}")
                        with tc.high_priority(offset=48):
                            for hh in range(2):
                                half = slice(hh * 64, hh * 64 + 64)
                                nc.tensor.matmul(s2[:, hh * 512 + mm_lo:hh * 512 + 512],
                                                 kT[half, pr, kt * 128:(kt + 1) * 128],
                                                 qT[half, pr, q0 + mm_lo:q0 + 512],
                                                 start=True, stop=True)
                        s2v = s2.rearrange("p (b n) -> p b n", b=2)
                        p2 = ppool.tile([128, 1024], F32R, tag="p", name=f"p{pr}_{qt}_{kt}")
                        p2v = p2.rearrange("p (b n) -> p b n", b=2)
                        with tc.high_priority(offset=96):
                            nc.scalar.activation(p2v[:, :, lo:512],
                                                 s2v[:, :, lo:512], EXP,
                                                 bias=keyb_t[:, kt:kt + 1], scale=0.125)
                        if c >= 0:
                            nc.vector.tensor_mul(p2v[:, :, lo:lo + 128],
                                                 p2v[:, :, lo:lo + 128],
                                                 dim0bc(tri_t, 2))
                        for hh in range(2):
                            nc.tensor.matmul(ys2[:, hh * 512 + lo:hh * 512 + 512],
                                             vA[:, 2 * pr + hh, kt, :],
                                             p2[:, hh * 512 + lo:hh * 512 + 512],
                                             start=(kt == 0), stop=(kt == nk - 1),
                                             skip_group_check=True)
                    def tail(pr=pr, qt=qt, q0=q0, ys2=ys2):
                        drow = dpool.tile([1, 1024], F32, tag="dc", name=f"dc{pr}_{qt}")
                        nc.vector.tensor_copy(drow, ys2[HD:HD + 1, :])
                        rec1 = dpool.tile([1, 1024], F32, tag="d", name=f"d{pr}_{qt}")
                        nc.vector.reciprocal_approx_fast(rec1, drow)
                        rec2 = rpool.tile([HD, 1024], F32, tag="r", name=f"r{pr}_{qt}")
                        nc.gpsimd.partition_broadcast(rec2, rec1)
                        for hh in range(2):
                            nc.vector.tensor_mul(yT[hh * 64:hh * 64 + 64, pr, q0:q0 + 512],
                                                 ys2[0:HD, hh * 512:hh * 512 + 512],
                                                 rec2[:, hh * 512:hh * 512 + 512])
                    # defer this iteration's denominator/normalize tail until after
                    # the next iteration's matmul loop so its DVE work doesn't
                    # delay the next wave of exps
                    if prev_tail is not None:
                        prev_tail()
                    prev_tail = tail
            prev_tail()

        # ---------------- Phase 3: out projection ----------------
        with (
            tc.tile_pool(name="ob", bufs=4) as opool,
            tc.tile_pool(name="pso", bufs=4, space="PSUM") as psop,
        ):
            for tt in range(16):
                for oh in range(2):
                    po = psop.tile([128, 512], F32, tag="o", name=f"po{tt}_{oh}")
                    for g in range(NP):
                        nc.tensor.matmul(po, yT[:, g, tt * 128:(tt + 1) * 128],
                                         wo_t[:, g, oh * 512:(oh + 1) * 512],
                                         start=(g == 0), stop=(g == NP - 1))
                    ob = opool.tile([128, 512], F32, tag="ob", name=f"ob{tt}_{oh}")
                    nc.vector.scalar_tensor_tensor(ob, po, qmt_t[:, tt:tt + 1],
                                                   borep[:, oh * 512:(oh + 1) * 512],
                                                   op0=MULT, op1=ADD)
                    nc.sync.dma_start(out=out[tt * 128:(tt + 1) * 128, oh * 512:(oh + 1) * 512],
                                      in_=ob)
        for p in (ypool, qkvpool, wopool, cpool):
            p.release()
    nc.compile()
    return nc


def _host_inputs(x, attention_mask, Wqkv, bqkv, Wo, bo):
    """Build the 8 per-core input maps."""
    x = np.asarray(x, dtype=np.float32)
    attention_mask = np.asarray(attention_mask)
    Wqkv = np.asarray(Wqkv, dtype=np.float32)
    bqkv = np.asarray(bqkv, dtype=np.float32)
    Wo = np.asarray(Wo, dtype=np.float32)
    bo = np.asarray(bo, dtype=np.float32)

    tri = (np.arange(128)[:, None] <= np.arange(128)[None, :]).astype(np.float32)
    zeros_bo = np.zeros_like(bo)
    in_maps = []
    for core in range(8):
        b = core // 2
        hg = core % 2
        cs = hg * 512
        m = attention_mask[b].astype(bool)
        keyb = np.where(m, 0.0, NEG).astype(np.float32).reshape(16, 128).T.copy()
        qmt = m.astype(np.float32).reshape(16, 128).T.copy()
        in_maps.append({
            "xt": np.ascontiguousarray(x[b].T),
            "wq": np.ascontiguousarray(Wqkv[cs:cs + 512, :].T),
            "wk": np.ascontiguousarray(Wqkv[D + cs:D + cs + 512, :].T),
            "wv": np.ascontiguousarray(Wqkv[2 * D + cs:2 * D + cs + 512, :].T),
            "wo": np.ascontiguousarray(Wo[:, cs:cs + 512].T),
            "bqt": np.ascontiguousarray(bqkv[cs:cs + 512].reshape(NP, 128).T),
            "bkt": np.ascontiguousarray(bqkv[D + cs:D + cs + 512].reshape(NP, 128).T),
            "bv": bqkv[2 * D + cs:2 * D + cs + 512].copy(),
            "bo": bo if hg == 0 else zeros_bo,
            "tri": tri,
            "keyb": keyb,
            "qmt": qmt,
        })
    return in_maps


def kernel(x, attention_mask, Wqkv, bqkv, Wo, bo, _trace=False, _trace_kwargs=None):
    if "nc" not in _CACHE:
        _CACHE["nc"] = _build()
    nc = _CACHE["nc"]
    in_maps = _host_inputs(x, attention_mask, Wqkv, bqkv, Wo, bo)
    kwargs = {}
    if _trace:
        kwargs["trace"] = True
        if _trace_kwargs:
            kwargs.update(_trace_kwargs)
    res = run_bass_kernel_spmd(nc, in_maps, core_ids=list(range(8)), **kwargs)
    _CACHE["last_result"] = res
    out = np.empty((B, T, D), dtype=np.float32)
    for b in range(B):
        out[b] = res.results[2 * b]["out"] + res.results[2 * b + 1]["out"]
    return out



# revision 3
# speedup vs baseline: 1.1760x; 1.1760x over previous
"""Causal self-attention (B=4, T=2048, D=1024, H=16) on 8 Trainium2 NeuronCores.

Sharding: data-parallel over batch (4) x tensor-parallel over heads (2 groups
of 8 heads) = 8 cores. Each core computes q/k/v projections for its 8 heads,
head-local attention, and a partial out-projection; the host sums the two
partials per batch element (the out_proj all-reduce).

v2 design (vs the fp32r baseline):
  - fp16 matmul pipeline: x, Wq, Wk resident in SBUF as fp16; q/k projections,
    S = k^T q, P@V and the out-projection all run on fp16 operands
    (1 cycle/row with no fp32r >=256-column constraint, so causal diagonal
    tiles shrink to their exact width).
  - V projection via fp8e4 DoubleRow matmuls (2 contraction k-tiles per
    instruction at 0.5 cycles/row): 3-term residual split
    x@Wv ~= xh@wvh + xl@(wvh/32) + xh@(wvl) keeps the error ~2e-3 while
    cutting the V matmul time 4x. Wv is pre-scaled by 32 on the host
    (folded back via qmt = mask/32 in the epilogue).
  - Transposed P@V: out[128q, 65] per (head, q-tile, k-tile) with the P^T
    tile as stationary and V (plus a ones column for the softmax
    denominator) as moving operand: 65 rows/tile instead of ~512.
  - y normalized token-major with per-partition reciprocals, transposed
    back to dim-major via PE transpose (paired heads, [128,128] tiles),
    then one DVE copy per 512 tokens into fp16 yT for the out-projection.
  - qk projections of head-pair pr+1 and the out-projection are woven into
    the attention loop of pr to fill PE bubbles left by the exp activation
    (ACT is the phase-2 bottleneck at ~0.83 ns/row).
"""
import numpy as np
import ml_dtypes

import concourse.bass as bass
import concourse.bacc as bacc
import concourse.tile as tile
from concourse import mybir
from concourse.bass_utils import run_bass_kernel_spmd

F32 = mybir.dt.float32
F16 = mybir.dt.float16
F8 = mybir.dt.float8e4
DR = mybir.MatmulPerfMode.DoubleRow
EXP = mybir.ActivationFunctionType.Exp
MULT = mybir.AluOpType.mult
ADD = mybir.AluOpType.add

B, T, D = 4, 2048, 1024
H = 16            # total heads
HD = 64           # head dim
HC = 8            # heads per core
NP = 4            # head pairs per core
NEG = -1.0e6      # additive mask value (exp -> 0)
NP8 = np.dtype(ml_dtypes.float8_e4m3fn)

_CACHE = {}


def _build():
    nc = bacc.Bacc("TRN2", target_bir_lowering=False, debug=False,
                   enable_asserts=False)

    def din(name, shape, dt=F32):
        return nc.dram_tensor(name, shape, dt, kind="ExternalInput").ap()

    xt16 = din("xt16", [D, T], F16)
    xh8 = din("xh8", [D, T], F8)
    xl8 = din("xl8", [D, T], F8)
    wq = din("wq", [D, 512], F16)
    wk = din("wk", [D, 512], F16)
    wvh = din("wvh", [D, 512], F8)
    wvh32 = din("wvh32", [D, 512], F8)
    wvl32 = din("wvl32", [D, 512], F8)
    wo = din("wo", [512, D], F16)
    bqt = din("bqt", [128, NP])
    bkt = din("bkt", [128, NP])
    bv = din("bv", [512])
    bo = din("bo", [D])
    tri = din("tri", [128, 128], F16)
    keyb = din("keyb", [128, 16])
    qmt = din("qmt", [128, 16])       # query mask / 32
    idm = din("idm", [128, 128], F16)
    out = nc.dram_tensor("out", [T, D], F32, kind="ExternalOutput").ap()

    def bcast128(ap):
        return bass.AP(tensor=ap.tensor, offset=ap.offset, ap=[[0, 128]] + ap.ap)

    def dim0bc(ap, n):
        # insert a stride-0 dim after the partition dim (broadcast)
        return bass.AP(tensor=ap.tensor, offset=ap.offset,
                       ap=[ap.ap[0], [0, n]] + ap.ap[1:])

    with tile.TileContext(nc) as tc:
        cpool = tc.alloc_tile_pool(name="const", bufs=1)
        tri_t = cpool.tile([128, 128], F16)
        idm_t = cpool.tile([128, 128], F16)
        keyb_t = cpool.tile([128, 16], F32)
        qmt_t = cpool.tile([128, 16], F32)
        bqt_t = cpool.tile([128, NP], F32)
        bkt_t = cpool.tile([128, NP], F32)
        bvrep = cpool.tile([128, 512], F32)
        borep = cpool.tile([128, D], F32)
        vone_f = cpool.tile([128, HC, 16], F16)
        nc.sync.dma_start(out=tri_t, in_=tri)
        nc.sync.dma_start(out=idm_t, in_=idm)
        nc.sync.dma_start(out=keyb_t, in_=keyb)
        nc.sync.dma_start(out=qmt_t, in_=qmt)
        nc.sync.dma_start(out=bqt_t, in_=bqt)
        nc.sync.dma_start(out=bkt_t, in_=bkt)
        nc.vector.memset(vone_f, 1.0)

        wpool = tc.alloc_tile_pool(name="wts", bufs=1)
        wq_t = wpool.tile([128, 8, 512], F16)
        wk_t = wpool.tile([128, 8, 512], F16)
        wo_t = wpool.tile([128, 4, D], F16)

        xpool = tc.alloc_tile_pool(name="xp", bufs=1)
        x16 = xpool.tile([128, 8, T], F16)

        qkvpool = tc.alloc_tile_pool(name="qkv", bufs=1)
        qT = qkvpool.tile([128, NP, T], F16, tag="qT")
        kT = qkvpool.tile([128, NP, T], F16, tag="kT")
        vA = qkvpool.tile([128, HC, 16, HD + 1], F16, tag="vA")
        nc.vector.tensor_copy(vA[:, :, :, HD], vone_f)

        ypool = tc.alloc_tile_pool(name="yT", bufs=1)
        yT = ypool.tile([128, NP, T], F16)

        # ---------------- prologue: V projection (fp8 DR) + q/k for pr=0 ----
        with (
            tc.tile_pool(name="x8p", bufs=1) as x8pool,
            tc.tile_pool(name="wv8", bufs=1) as wv8pool,
            tc.tile_pool(name="psv", bufs=3, space="PSUM") as psv,
        ):
            xh_t = x8pool.tile([128, 8, T], F8, tag="xh")
            xl_t = x8pool.tile([128, 8, T], F8, tag="xl")
            wvh_t = wv8pool.tile([128, 8, 512], F8, tag="wvh")
            wvh32_t = wv8pool.tile([128, 8, 512], F8, tag="wvh32")
            wvl32_t = wv8pool.tile([128, 8, 512], F8, tag="wvl32")

            nc.gpsimd.dma_start(out=wvh_t, in_=wvh.rearrange("(k p) n -> p k n", p=128))
            nc.gpsimd.dma_start(out=wvh32_t, in_=wvh32.rearrange("(k p) n -> p k n", p=128))
            nc.gpsimd.dma_start(out=wvl32_t, in_=wvl32.rearrange("(k p) n -> p k n", p=128))
            # x fp8 hi/lo: 8 kd x 2 halves each, spread over the four DMA queues
            engs = [nc.sync, nc.scalar, nc.gpsimd, nc.sync]
            for half in range(2):
                h0 = half * 1024
                for kd in range(8):
                    eng = engs[kd % 4]
                    eng.dma_start(out=xh_t[:, kd, h0:h0 + 1024],
                                  in_=xh8[kd * 128:(kd + 1) * 128, h0:h0 + 1024])
                    eng = engs[(kd + 2) % 4]
                    eng.dma_start(out=xl_t[:, kd, h0:h0 + 1024],
                                  in_=xl8[kd * 128:(kd + 1) * 128, h0:h0 + 1024])
            # x fp16 (for q/k moving operand), then weights
            for kd in range(8):
                eng = engs[kd % 2]
                eng.dma_start(out=x16[:, kd, :], in_=xt16[kd * 128:(kd + 1) * 128, :])
            nc.gpsimd.dma_start(out=wq_t, in_=wq.rearrange("(k p) n -> p k n", p=128))
            nc.gpsimd.dma_start(out=wk_t, in_=wk.rearrange("(k p) n -> p k n", p=128))
            nc.sync.dma_start(out=bvrep, in_=bcast128(bv))
            nc.sync.dma_start(out=borep, in_=bcast128(bo))
            nc.gpsimd.dma_start(out=wo_t, in_=wo.rearrange("(g p) n -> p g n", p=128))

            # V projection: token-major, 3-term fp8 DoubleRow
            for tt in range(16):
                ps = psv.tile([128, 512], F32, tag="pv", name=f"pv{tt}")
                first = True
                for stat_t, mov_t in ((xh_t, wvh_t), (xl_t, wvh32_t), (xh_t, wvl32_t)):
                    for j in range(4):
                        nc.tensor.matmul(
                            ps, stat_t[:, 2 * j:2 * j + 2, tt * 128:(tt + 1) * 128],
                            mov_t[:, 2 * j:2 * j + 2, :],
                            start=first, stop=(stat_t is xh_t and mov_t is wvl32_t and j == 3),
                            perf_mode=DR)
                        first = False
                nc.vector.tensor_add(vA[:, :, tt, 0:HD],
                                     ps.rearrange("p (h d) -> p h d", h=HC),
                                     bvrep.rearrange("p (h d) -> p h d", h=HC))

            # q/k projections for pr=0
            for wt, bias_t, outT in ((wq_t, bqt_t, qT), (wk_t, bkt_t, kT)):
                for cch in range(4):
                    ps = psv.tile([128, 512], F32, tag="pv", name=f"p0{wt is wk_t}_{cch}")
                    for kd in range(8):
                        nc.tensor.matmul(ps, wt[:, kd, 0:128],
                                         x16[:, kd, cch * 512:(cch + 1) * 512],
                                         start=(kd == 0), stop=(kd == 7))
                    nc.vector.tensor_scalar_add(outT[:, 0, cch * 512:(cch + 1) * 512],
                                                ps, bias_t[:, 0:1])

        # ---------------- attention + interleaved qk(pr+1) / out-proj -------
        with (
            tc.tile_pool(name="ptile", bufs=6) as ppool,
            tc.tile_pool(name="ynp", bufs=2) as ynpool,
            tc.tile_pool(name="recp", bufs=2) as recpool,
            tc.tile_pool(name="obi", bufs=2) as obipool,
            tc.tile_pool(name="pss", bufs=2, space="PSUM") as pss,
            tc.tile_pool(name="psy", bufs=1, space="PSUM") as psy,
            tc.tile_pool(name="ptr", bufs=1, space="PSUM") as ptr,
            tc.tile_pool(name="paux", bufs=1, space="PSUM") as paux,
        ):
            def qk_steps(pr):
                """Generator of matmul/bias thunks for q/k projection of pr."""
                for wt, bias_t, outT in ((wq_t, bqt_t, qT), (wk_t, bkt_t, kT)):
                    for cch in range(4):
                        ps = [None]

                        def mk(kd, wt=wt, cch=cch, ps=ps):
                            def go():
                                if kd == 0:
                                    ps[0] = paux.tile([128, 512], F32, tag="aux",
                                                      name=f"qk{pr}_{wt is wk_t}_{cch}")
                                nc.tensor.matmul(ps[0], wt[:, kd, pr * 128:(pr + 1) * 128],
                                                 x16[:, kd, cch * 512:(cch + 1) * 512],
                                                 start=(kd == 0), stop=(kd == 7))
                            return go
                        for kd in range(8):
                            yield mk(kd)

                        def bias(wt=wt, bias_t=bias_t, outT=outT, cch=cch, ps=ps):
                            nc.vector.tensor_scalar_add(
                                outT[:, pr, cch * 512:(cch + 1) * 512], ps[0],
                                bias_t[:, pr:pr + 1])
                        yield bias

            def oproj_steps(tt_list):
                """Generator of out-projection thunks for token tiles tt_list."""
                for tt in tt_list:
                    for oh in range(2):
                        po = [None]

                        def mk(g, tt=tt, oh=oh, po=po):
                            def go():
                                if g == 0:
                                    po[0] = paux.tile([128, 512], F32, tag="aux",
                                                      name=f"po{tt}_{oh}")
                                nc.tensor.matmul(po[0], yT[:, g, tt * 128:(tt + 1) * 128],
                                                 wo_t[:, g, oh * 512:(oh + 1) * 512],
                                                 start=(g == 0), stop=(g == NP - 1))
                            return go
                        for g in range(NP):
                            yield mk(g)

                        def fin(tt=tt, oh=oh, po=po):
                            ob = obipool.tile([128, 512], F32, tag="ob",
                                              name=f"ob{tt}_{oh}")
                            nc.vector.scalar_tensor_tensor(
                                ob, po[0], qmt_t[:, tt:tt + 1],
                                borep[:, oh * 512:(oh + 1) * 512], op0=MULT, op1=ADD)
                            nc.sync.dma_start(
                                out=out[tt * 128:(tt + 1) * 128, oh * 512:(oh + 1) * 512],
                                in_=ob)
                        yield fin

            done_oproj = []

            def fill(n, queue):
                for _ in range(n):
                    step = next(queue, None)
                    if step is None:
                        return
                    step()

            prev_tail = None
            for pr in range(NP):
                if pr < NP - 1:
                    queue = qk_steps(pr + 1)
                else:
                    queue = iter(())
                for qt in range(4):
                    q0 = qt * 512
                    nk = qt * 4 + 4
                    if pr == NP - 1 and qt > 0:
                        # out-proj of token block qt-1 (yT complete for it)
                        tts = list(range((qt - 1) * 4, qt * 4))
                        done_oproj += tts
                        queue = oproj_steps(tts)
                    ys = psy.tile([128, 2, 512], F32, tag="ys", name=f"ys{pr}_{qt}")
                    for kt in range(nk):
                        c = kt * 128 - q0
                        lo = max(c, 0)
                        smin = max(c // 128, 0)
                        s2 = pss.tile([128, 1024], F32, tag="s", name=f"s{pr}_{qt}_{kt}")
                        with tc.high_priority(offset=48):
                            for hh in range(2):
                                half = slice(hh * 64, hh * 64 + 64)
                                nc.tensor.matmul(s2[:, hh * 512 + lo:hh * 512 + 512],
                                                 kT[half, pr, kt * 128:(kt + 1) * 128],
                                                 qT[half, pr, q0 + lo:q0 + 512],
                                                 start=True, stop=True)
                        s2v = s2.rearrange("p (b n) -> p b n", b=2)
                        p2 = ppool.tile([128, 1024], F16, tag="p", name=f"p{pr}_{qt}_{kt}")
                        p2v = p2.rearrange("p (b n) -> p b n", b=2)
                        with tc.high_priority(offset=96):
                            nc.scalar.activation(p2v[:, :, lo:512],
                                                 s2v[:, :, lo:512], EXP,
                                                 bias=keyb_t[:, kt:kt + 1], scale=0.125)
                        if c >= 0:
                            nc.vector.tensor_mul(p2v[:, :, lo:lo + 128],
                                                 p2v[:, :, lo:lo + 128],
                                                 dim0bc(tri_t, 2))
                        if prev_tail is not None and kt == 1:
                            prev_tail()
                            prev_tail = None
                        for hh in range(2):
                            for s in range(smin, 4):
                                nc.tensor.matmul(
                                    ys[:, hh, s * 65:s * 65 + 65],
                                    p2[:, hh * 512 + s * 128:hh * 512 + (s + 1) * 128],
                                    vA[:, 2 * pr + hh, kt, :],
                                    start=(kt == 0 and s == 0),
                                    stop=(kt == qt * 4 + s),
                                    skip_group_check=True)
                        fill(2, queue)

                    def tail(pr=pr, qt=qt, q0=q0, ys=ys):
                        denS = recpool.tile([128, 8], F32, tag="dn", name=f"dn{pr}_{qt}")
                        den_ap = bass.AP(tensor=ys.tensor, offset=ys.offset + 64,
                                         ap=[ys.ap[0], [512, 2], [65, 4]])
                        nc.vector.tensor_copy(denS, den_ap)
                        rec = recpool.tile([128, 8], F32, tag="rc", name=f"rc{pr}_{qt}")
                        nc.vector.reciprocal_approx_fast(rec, denS)
                        yn = ynpool.tile([128, 4, 128], F16, tag="yn", name=f"yn{pr}_{qt}")
                        tr = ptr.tile([128, 4, 128], F16, tag="tr", name=f"tr{pr}_{qt}")
                        for s in range(4):
                            out_ap = bass.AP(tensor=yn.tensor, offset=yn.offset + s * 128,
                                             ap=[yn.ap[0], [64, 2], [1, 64]])
                            in0 = bass.AP(tensor=ys.tensor, offset=ys.offset + s * 65,
                                          ap=[ys.ap[0], [512, 2], [1, 64]])
                            in1 = bass.AP(tensor=rec.tensor, offset=rec.offset + s,
                                          ap=[rec.ap[0], [4, 2], [0, 64]])
                            nc.vector.tensor_tensor(out_ap, in0, in1, op=MULT)
                        for s in range(4):
                            nc.tensor.matmul(tr[:, s, :], yn[:, s, :], idm_t,
                                             is_transpose=True)
                        nc.vector.tensor_copy(yT[:, pr, q0:q0 + 512],
                                              tr.rearrange("p s n -> p (s n)"))
                    if prev_tail is not None:
                        prev_tail()
                    prev_tail = tail
                # drain any leftover interleave work at the end of this pr
                fill(1000, queue)
            prev_tail()

        # ---------------- out-projection tail --------------------------------
        with (
            tc.tile_pool(name="ob", bufs=4) as opool,
            tc.tile_pool(name="pso", bufs=4, space="PSUM") as psop,
        ):
            for tt in range(16):
                if tt in done_oproj:
                    continue
                for oh in range(2):
                    po = psop.tile([128, 512], F32, tag="o", name=f"pot{tt}_{oh}")
                    for g in range(NP):
                        nc.tensor.matmul(po, yT[:, g, tt * 128:(tt + 1) * 128],
                                         wo_t[:, g, oh * 512:(oh + 1) * 512],
                                         start=(g == 0), stop=(g == NP - 1))
                    ob = opool.tile([128, 512], F32, tag="obt", name=f"obt{tt}_{oh}")
                    nc.vector.scalar_tensor_tensor(ob, po, qmt_t[:, tt:tt + 1],
                                                   borep[:, oh * 512:(oh + 1) * 512],
                                                   op0=MULT, op1=ADD)
                    nc.sync.dma_start(out=out[tt * 128:(tt + 1) * 128,
                                              oh * 512:(oh + 1) * 512],
                                      in_=ob)
        for p in (ypool, qkvpool, xpool, wpool, cpool):
            p.release()
    nc.compile()
    return nc


def _f8(a):
    return np.asarray(a, dtype=np.float32).astype(NP8)


def _host_inputs(x, attention_mask, Wqkv, bqkv, Wo, bo):
    """Build the 8 per-core input maps."""
    x = np.asarray(x, dtype=np.float32)
    attention_mask = np.asarray(attention_mask)
    Wqkv = np.asarray(Wqkv, dtype=np.float32)
    bqkv = np.asarray(bqkv, dtype=np.float32)
    Wo = np.asarray(Wo, dtype=np.float32)
    bo = np.asarray(bo, dtype=np.float32)

    tri = (np.arange(128)[:, None] <= np.arange(128)[None, :]).astype(np.float16)
    idm = np.eye(128, dtype=np.float16)
    zeros_bo = np.zeros_like(bo)
    in_maps = []
    for core in range(8):
        b = core // 2
        hg = core % 2
        cs = hg * 512
        m = attention_mask[b].astype(bool)
        keyb = np.where(m, 0.0, NEG).astype(np.float32).reshape(16, 128).T.copy()
        qmt = (m.astype(np.float32) / 32.0).reshape(16, 128).T.copy()
        xt = np.ascontiguousarray(x[b].T)
        xh = xt.astype(NP8)
        xl = ((xt - xh.astype(np.float32)) * 32.0).astype(NP8)
        wvs = np.ascontiguousarray(Wqkv[2 * D + cs:2 * D + cs + 512, :].T) * 32.0
        wvh = wvs.astype(NP8)
        wvh32 = (wvh.astype(np.float32) / 32.0).astype(NP8)
        wvl32 = (wvs - wvh.astype(np.float32)).astype(NP8)
        in_maps.append({
            "xt16": xt.astype(np.float16),
            "xh8": xh,
            "xl8": xl,
            "wq": np.ascontiguousarray(Wqkv[cs:cs + 512, :].T).astype(np.float16),
            "wk": np.ascontiguousarray(Wqkv[D + cs:D + cs + 512, :].T).astype(np.float16),
            "wvh": wvh,
            "wvh32": wvh32,
            "wvl32": wvl32,
            "wo": np.ascontiguousarray(Wo[:, cs:cs + 512].T).astype(np.float16),
            "bqt": np.ascontiguousarray(bqkv[cs:cs + 512].reshape(NP, 128).T),
            "bkt": np.ascontiguousarray(bqkv[D + cs:D + cs + 512].reshape(NP, 128).T),
            "bv": bqkv[2 * D + cs:2 * D + cs + 512].copy() * 32.0,
            "bo": bo if hg == 0 else zeros_bo,
            "tri": tri,
            "keyb": keyb,
            "qmt": qmt,
            "idm": idm,
        })
    return in_maps


def kernel(x, attention_mask, Wqkv, bqkv, Wo, bo, _trace=False, _trace_kwargs=None):
    if "nc" not in _CACHE:
        _CACHE["nc"] = _build()
    nc = _CACHE["nc"]
    in_maps = _host_inputs(x, attention_mask, Wqkv, bqkv, Wo, bo)
    kwargs = {}
    if _trace:
        kwargs["trace"] = True
        if _trace_kwargs:
            kwargs.update(_trace_kwargs)
    res = run_bass_kernel_spmd(nc, in_maps, core_ids=list(range(8)), **kwargs)
    _CACHE["last_result"] = res
    out = np.empty((B, T, D), dtype=np.float32)
    for b in range(B):
        out[b] = res.results[2 * b]["out"] + res.results[2 * b + 1]["out"]
    return out


# revision 15
# speedup vs baseline: 1.2169x; 1.0348x over previous
"""Causal self-attention (B=4, T=2048, D=1024, H=16) on 8 Trainium2 NeuronCores.

Sharding: data-parallel over batch (4) x tensor-parallel over heads (2 groups
of 8 heads) = 8 cores. Each core computes q/k/v projections for its 8 heads,
head-local attention, and a partial out-projection; the host sums the two
partials per batch element (the out_proj all-reduce).

v2 design (vs the fp32r baseline):
  - fp16 matmul pipeline: x, Wq, Wk resident in SBUF as fp16; q/k projections,
    S = k^T q, P@V and the out-projection all run on fp16 operands
    (1 cycle/row with no fp32r >=256-column constraint, so causal diagonal
    tiles shrink to their exact width).
  - V projection via fp8e4 DoubleRow matmuls (2 contraction k-tiles per
    instruction at 0.5 cycles/row): 3-term residual split
    x@Wv ~= xh@wvh + xl@(wvh/32) + xh@(wvl) keeps the error ~2e-3 while
    cutting the V matmul time 4x. Wv is pre-scaled by 32 on the host
    (folded back via qmt = mask/32 in the epilogue).
  - Transposed P@V: out[128q, 65] per (head, q-tile, k-tile) with the P^T
    tile as stationary and V (plus a ones column for the softmax
    denominator) as moving operand: 65 rows/tile instead of ~512.
  - y normalized token-major with per-partition reciprocals, transposed
    back to dim-major via PE transpose (paired heads, [128,128] tiles),
    then one DVE copy per 512 tokens into fp16 yT for the out-projection.
  - qk projections of head-pair pr+1 and the out-projection are woven into
    the attention loop of pr to fill PE bubbles left by the exp activation
    (ACT is the phase-2 bottleneck at ~0.83 ns/row).
"""
import numpy as np
import ml_dtypes

import concourse.bass as bass
import concourse.bacc as bacc
import concourse.tile as tile
from concourse import mybir
from concourse.bass_utils import run_bass_kernel_spmd

F32 = mybir.dt.float32
F16 = mybir.dt.float16
F8 = mybir.dt.float8e4
DR = mybir.MatmulPerfMode.DoubleRow
EXP = mybir.ActivationFunctionType.Exp
MULT = mybir.AluOpType.mult
ADD = mybir.AluOpType.add

B, T, D = 4, 2048, 1024
H = 16            # total heads
HD = 64           # head dim
HC = 8            # heads per core
NP = 4            # head pairs per core
NEG = -1.0e6      # additive mask value (exp -> 0)
NP8 = np.dtype(ml_dtypes.float8_e4m3fn)

_CACHE = {}


def _build():
    nc = bacc.Bacc("TRN2", target_bir_lowering=False, debug=False,
                   enable_asserts=False)

    def din(name, shape, dt=F32):
        return nc.dram_tensor(name, shape, dt, kind="ExternalInput").ap()

    xh8 = din("xh8", [D, T], F8)
    xl8 = din("xl8", [D, T], F8)
    wqh = din("wqh", [D, 512], F8)
    wqh32 = din("wqh32", [D, 512], F8)
    wql32 = din("wql32", [D, 512], F8)
    wkh = din("wkh", [D, 512], F8)
    wkh32 = din("wkh32", [D, 512], F8)
    wkl32 = din("wkl32", [D, 512], F8)
    wvh = din("wvh", [D, 512], F8)
    wvh32 = din("wvh32", [D, 512], F8)
    wvl32 = din("wvl32", [D, 512], F8)
    wo = din("wo", [512, D], F16)
    bqt = din("bqt", [128, NP])
    bkt = din("bkt", [128, NP])
    bv = din("bv", [512])
    bo = din("bo", [D])
    tri = din("tri", [128, 128], F16)
    keyb = din("keyb", [128, 16])
    qmt = din("qmt", [128, 16])       # query mask / 32
    idm = din("idm", [128, 128], F16)
    out = nc.dram_tensor("out", [T, D], F32, kind="ExternalOutput").ap()

    def bcast128(ap):
        return bass.AP(tensor=ap.tensor, offset=ap.offset, ap=[[0, 128]] + ap.ap)

    def dim0bc(ap, n):
        # insert a stride-0 dim after the partition dim (broadcast)
        return bass.AP(tensor=ap.tensor, offset=ap.offset,
                       ap=[ap.ap[0], [0, n]] + ap.ap[1:])

    with tile.TileContext(nc) as tc:
        cpool = tc.alloc_tile_pool(name="const", bufs=1)
        tri_t = cpool.tile([128, 128], F16)
        idm_t = cpool.tile([128, 128], F16)
        keyb_t = cpool.tile([128, 16], F32)
        qmt_t = cpool.tile([128, 16], F32)
        bqt_t = cpool.tile([128, NP], F32)
        bkt_t = cpool.tile([128, NP], F32)
        bvrep = cpool.tile([128, 512], F32)
        borep = cpool.tile([128, D], F32)
        vone_f = cpool.tile([128, HC, 16], F16)
        nc.sync.dma_start(out=tri_t, in_=tri)
        nc.sync.dma_start(out=idm_t, in_=idm)
        nc.sync.dma_start(out=keyb_t, in_=keyb)
        nc.sync.dma_start(out=qmt_t, in_=qmt)
        nc.sync.dma_start(out=bqt_t, in_=bqt)
        nc.sync.dma_start(out=bkt_t, in_=bkt)
        nc.vector.memset(vone_f, 1.0)

        wpool = tc.alloc_tile_pool(name="wts", bufs=1)
        wq_trio = [wpool.tile([128, 8, 512], F8, tag=f"wq{i}", name=f"wq{i}")
                   for i in range(3)]
        wk_trio = [wpool.tile([128, 8, 512], F8, tag=f"wk{i}", name=f"wk{i}")
                   for i in range(3)]
        wo_t = wpool.tile([128, 4, D], F16)

        xpool = tc.alloc_tile_pool(name="xp", bufs=1)
        xh_t = xpool.tile([128, 8, T], F8, tag="xh")
        xl_t = xpool.tile([128, 8, T], F8, tag="xl")

        qkvpool = tc.alloc_tile_pool(name="qkv", bufs=1)
        qT = qkvpool.tile([128, NP, T], F16, tag="qT")
        kT = qkvpool.tile([128, NP, T], F16, tag="kT")
        vA = qkvpool.tile([128, HC, 16, HD + 1], F16, tag="vA")
        nc.vector.tensor_copy(vA[:, :, :, HD], vone_f)

        ypool = tc.alloc_tile_pool(name="yT", bufs=1)
        yT = ypool.tile([128, NP, T], F16)

        def qk_chunks(pr, wt_trio, bias_t, outT, cch, pool):
            """One q/k projection psum group (3-term fp8 DR) + bias-add, split
            into 4 emission chunks (~430ns of PE work each)."""
            ps = [None]
            terms = ((wt_trio[0], xh_t), (wt_trio[1], xl_t), (wt_trio[2], xh_t))

            def mm_term(term):
                st, mv = terms[term]
                if term == 0:
                    ps[0] = pool.tile([128, 512], F32, tag=pool._qk_tag,
                                      name=f"qk{pr}_{outT is kT}_{cch}")
                for j in range(4):
                    nc.tensor.matmul(
                        ps[0], st[:, 2 * j:2 * j + 2, pr * 128:(pr + 1) * 128],
                        mv[:, 2 * j:2 * j + 2, cch * 512:(cch + 1) * 512],
                        start=(term == 0 and j == 0), stop=(term == 2 and j == 3),
                        perf_mode=DR)

            def bias():
                nc.vector.tensor_scalar_add(outT[:, pr, cch * 512:(cch + 1) * 512],
                                            ps[0], bias_t[:, pr:pr + 1])
            return [lambda t=t: mm_term(t) for t in range(3)] + [bias]

        # ---------------- prologue: V projection (fp8 DR) + q/k for pr=0 ----
        with (
            tc.tile_pool(name="wv8", bufs=1) as wv8pool,
            tc.tile_pool(name="psv", bufs=3, space="PSUM") as psv,
        ):
            psv._qk_tag = "pv"
            wvh_t = wv8pool.tile([128, 8, 512], F8, tag="wvh")
            wvh32_t = wv8pool.tile([128, 8, 512], F8, tag="wvh32")
            wvl32_t = wv8pool.tile([128, 8, 512], F8, tag="wvl32")

            # DMA plan, ordered by first use. scalar queue: the V weight trio;
            # sync queue: x8 chunks; gpsimd: consts + q/k weight trios.
            nc.scalar.dma_start(out=wvh_t, in_=wvh.rearrange("(k p) n -> p k n", p=128))
            nc.scalar.dma_start(out=wvh32_t, in_=wvh32.rearrange("(k p) n -> p k n", p=128))
            nc.scalar.dma_start(out=wvl32_t, in_=wvl32.rearrange("(k p) n -> p k n", p=128))
            for c0, c1, eng in ((0, 512, nc.sync), (512, 1024, nc.sync),
                                (1024, 1536, nc.scalar), (1536, 2048, nc.sync)):
                eng.dma_start(out=xh_t[:, :, c0:c1],
                              in_=xh8[:, c0:c1].rearrange("(k p) t -> p k t", p=128))
                eng.dma_start(out=xl_t[:, :, c0:c1],
                              in_=xl8[:, c0:c1].rearrange("(k p) t -> p k t", p=128))
            for i, w in enumerate((wqh, wqh32, wql32)):
                nc.gpsimd.dma_start(out=wq_trio[i], in_=w.rearrange("(k p) n -> p k n", p=128))
            for i, w in enumerate((wkh, wkh32, wkl32)):
                nc.gpsimd.dma_start(out=wk_trio[i], in_=w.rearrange("(k p) n -> p k n", p=128))
            nc.sync.dma_start(out=bvrep, in_=bcast128(bv))
            nc.scalar.dma_start(out=borep, in_=bcast128(bo))
            nc.gpsimd.dma_start(out=wo_t, in_=wo.rearrange("(g p) n -> p g n", p=128))

            # V projection: token-major, 3-term fp8 DoubleRow
            for tt in range(16):
                ps = psv.tile([128, 512], F32, tag="pv", name=f"pv{tt}")
                first = True
                for stat_t, mov_t in ((xh_t, wvh_t), (xl_t, wvh32_t), (xh_t, wvl32_t)):
                    for j in range(4):
                        nc.tensor.matmul(
                            ps, stat_t[:, 2 * j:2 * j + 2, tt * 128:(tt + 1) * 128],
                            mov_t[:, 2 * j:2 * j + 2, :],
                            start=first, stop=(stat_t is xh_t and mov_t is wvl32_t and j == 3),
                            perf_mode=DR)
                        first = False
                nc.vector.tensor_add(vA[:, :, tt, 0:HD],
                                     ps.rearrange("p (h d) -> p h d", h=HC),
                                     bvrep.rearrange("p (h d) -> p h d", h=HC))

            # q/k projections for pr=0, token chunk 0 only (the rest is woven
            # into the attention loop of pr=0)
            for wt_trio, bias_t, outT in ((wq_trio, bqt_t, qT), (wk_trio, bkt_t, kT)):
                for step in qk_chunks(0, wt_trio, bias_t, outT, 0, psv):
                    step()

        # ---------------- attention + interleaved qk(pr+1) / out-proj -------
        with (
            tc.tile_pool(name="ptile", bufs=6) as ppool,
            tc.tile_pool(name="ynp", bufs=2) as ynpool,
            tc.tile_pool(name="recp", bufs=2) as recpool,
            tc.tile_pool(name="obi", bufs=2) as obipool,
            tc.tile_pool(name="pss", bufs=2, space="PSUM") as pss,
            tc.tile_pool(name="psy", bufs=1, space="PSUM") as psy,
            tc.tile_pool(name="ptr", bufs=1, space="PSUM") as ptr,
            tc.tile_pool(name="paux", bufs=1, space="PSUM") as paux,
        ):
            paux._qk_tag = "aux"

            def qk_steps(pr, cch_list):
                """q/k projection step thunks for head-pair pr, given chunks."""
                for cch in cch_list:
                    for wt_trio, bias_t, outT in ((wq_trio, bqt_t, qT),
                                                  (wk_trio, bkt_t, kT)):
                        yield from qk_chunks(pr, wt_trio, bias_t, outT, cch, paux)

            done_oproj = []

            def oproj_steps(tt_list):
                """Out-projection step thunks for token tiles tt_list."""
                for tt in tt_list:
                    for oh in range(2):
                        po = [None]

                        def mk(gg, tt=tt, oh=oh, po=po):
                            def go():
                                if gg == 0:
                                    po[0] = paux.tile([128, 512], F32, tag="aux",
                                                      name=f"po{tt}_{oh}")
                                for g in (gg, gg + 1):
                                    nc.tensor.matmul(
                                        po[0], yT[:, g, tt * 128:(tt + 1) * 128],
                                        wo_t[:, g, oh * 512:(oh + 1) * 512],
                                        start=(g == 0), stop=(g == NP - 1))
                            return go
                        yield mk(0)
                        yield mk(2)

                        def fin(tt=tt, oh=oh, po=po):
                            ob = obipool.tile([128, 512], F32, tag="ob",
                                              name=f"ob{tt}_{oh}")
                            nc.vector.scalar_tensor_tensor(
                                ob, po[0], qmt_t[:, tt:tt + 1],
                                borep[:, oh * 512:(oh + 1) * 512], op0=MULT, op1=ADD)
                            nc.sync.dma_start(
                                out=out[tt * 128:(tt + 1) * 128,
                                        oh * 512:(oh + 1) * 512],
                                in_=ob)
                            if oh == 1:
                                done_oproj.append(tt)
                        yield fin

            class Filler:
                def __init__(self, queue):
                    self.queue = queue
                    self.consumed = 0

                def fill(self, n):
                    for _ in range(n):
                        step = next(self.queue, None)
                        if step is None:
                            return
                        step()
                        self.consumed += 1

                def force(self, upto):
                    self.fill(max(0, upto - self.consumed))

            import itertools
            prev_tail = None
            for pr in range(NP):
                if pr == 0:
                    # remaining qk chunks of pr=0, then all of pr=1
                    fl = Filler(itertools.chain(qk_steps(0, [1, 2, 3]),
                                                qk_steps(1, [0, 1, 2, 3])))
                    rate = 2
                elif pr < NP - 1:
                    fl = Filler(qk_steps(pr + 1, [0, 1, 2, 3]))
                    rate = 1
                else:
                    fl = Filler(iter(()))
                    rate = 3
                for qt in range(4):
                    q0 = qt * 512
                    nk = qt * 4 + 4
                    if pr == 0 and qt > 0:
                        # pr0 still needs its own q/k chunk qt before using it
                        fl.force(qt * 8)
                    if pr == NP - 1 and qt > 0:
                        # out-proj of token block qt-1 (yT complete for it)
                        tts = list(range((qt - 1) * 4, qt * 4))
                        fl.queue = itertools.chain(fl.queue, oproj_steps(tts))
                    ys = psy.tile([128, 2, 512], F32, tag="ys", name=f"ys{pr}_{qt}")
                    for kt in range(nk):
                        c = kt * 128 - q0
                        lo = max(c, 0)
                        smin = max(c // 128, 0)
                        s2 = pss.tile([128, 1024], F32, tag="s", name=f"s{pr}_{qt}_{kt}")
                        with tc.high_priority(offset=48):
                            for hh in range(2):
                                half = slice(hh * 64, hh * 64 + 64)
                                nc.tensor.matmul(s2[:, hh * 512 + lo:hh * 512 + 512],
                                                 kT[half, pr, kt * 128:(kt + 1) * 128],
                                                 qT[half, pr, q0 + lo:q0 + 512],
                                                 start=True, stop=True)
                        s2v = s2.rearrange("p (b n) -> p b n", b=2)
                        p2 = ppool.tile([128, 1024], F16, tag="p", name=f"p{pr}_{qt}_{kt}")
                        p2v = p2.rearrange("p (b n) -> p b n", b=2)
                        with tc.high_priority(offset=96):
                            nc.scalar.activation(p2v[:, :, lo:512],
                                                 s2v[:, :, lo:512], EXP,
                                                 bias=keyb_t[:, kt:kt + 1],
                                                 scale=0.125 / 1024.0)
                        if c >= 0:
                            nc.vector.tensor_mul(p2v[:, :, lo:lo + 128],
                                                 p2v[:, :, lo:lo + 128],
                                                 dim0bc(tri_t, 2))
                        if prev_tail is not None and kt == 1:
                            prev_tail()
                            prev_tail = None
                        for hh in range(2):
                            for s in range(smin, 4):
                                nc.tensor.matmul(
                                    ys[:, hh, s * 65:s * 65 + 65],
                                    p2[:, hh * 512 + s * 128:hh * 512 + (s + 1) * 128],
                                    vA[:, 2 * pr + hh, kt, :],
                                    start=(kt == 0 and s == 0),
                                    stop=(kt == qt * 4 + s),
                                    skip_group_check=True)
                        fl.fill(rate)

                    def tail(pr=pr, qt=qt, q0=q0, ys=ys):
                        denS = recpool.tile([128, 8], F32, tag="dn", name=f"dn{pr}_{qt}")
                        den_ap = bass.AP(tensor=ys.tensor, offset=ys.offset + 64,
                                         ap=[ys.ap[0], [512, 2], [65, 4]])
                        nc.vector.tensor_copy(denS, den_ap)
                        rec = recpool.tile([128, 8], F32, tag="rc", name=f"rc{pr}_{qt}")
                        nc.vector.reciprocal_approx_fast(rec, denS)
                        yn = ynpool.tile([128, 4, 128], F16, tag="yn", name=f"yn{pr}_{qt}")
                        tr = ptr.tile([128, 4, 128], F16, tag="tr", name=f"tr{pr}_{qt}")
                        for s in range(4):
                            out_ap = bass.AP(tensor=yn.tensor, offset=yn.offset + s * 128,
                                             ap=[yn.ap[0], [64, 2], [1, 64]])
                            in0 = bass.AP(tensor=ys.tensor, offset=ys.offset + s * 65,
                                          ap=[ys.ap[0], [512, 2], [1, 64]])
                            in1 = bass.AP(tensor=rec.tensor, offset=rec.offset + s,
                                          ap=[rec.ap[0], [4, 2], [0, 64]])
                            nc.vector.tensor_tensor(out_ap, in0, in1, op=MULT)
                        for s in range(4):
                            nc.tensor.matmul(tr[:, s, :], yn[:, s, :], idm_t,
                                             is_transpose=True)
                        nc.vector.tensor_copy(yT[:, pr, q0:q0 + 512],
                                              tr.rearrange("p s n -> p (s n)"))
                    if prev_tail is not None:
                        prev_tail()
                    prev_tail = tail
                # qk of the next pr must be complete before its attention
                if pr < NP - 1:
                    fl.fill(1000)
            prev_tail()

        # ---------------- out-projection tail --------------------------------
        with (
            tc.tile_pool(name="ob", bufs=4) as opool,
            tc.tile_pool(name="pso", bufs=4, space="PSUM") as psop,
        ):
            for tt in range(16):
                if tt in done_oproj:
                    continue
                for oh in range(2):
                    po = psop.tile([128, 512], F32, tag="o", name=f"pot{tt}_{oh}")
                    for g in range(NP):
                        nc.tensor.matmul(po, yT[:, g, tt * 128:(tt + 1) * 128],
                                         wo_t[:, g, oh * 512:(oh + 1) * 512],
                                         start=(g == 0), stop=(g == NP - 1))
                    ob = opool.tile([128, 512], F32, tag="obt", name=f"obt{tt}_{oh}")
                    nc.vector.scalar_tensor_tensor(ob, po, qmt_t[:, tt:tt + 1],
                                                   borep[:, oh * 512:(oh + 1) * 512],
                                                   op0=MULT, op1=ADD)
                    nc.sync.dma_start(out=out[tt * 128:(tt + 1) * 128,
                                              oh * 512:(oh + 1) * 512],
                                      in_=ob)
        for p in (ypool, qkvpool, xpool, wpool, cpool):
            p.release()
    nc.compile()
    return nc


def _f8(a):
    return np.asarray(a, dtype=np.float32).astype(NP8)


def _host_inputs(x, attention_mask, Wqkv, bqkv, Wo, bo):
    """Build the 8 per-core input maps."""
    x = np.asarray(x, dtype=np.float32)
    attention_mask = np.asarray(attention_mask)
    Wqkv = np.asarray(Wqkv, dtype=np.float32)
    bqkv = np.asarray(bqkv, dtype=np.float32)
    Wo = np.asarray(Wo, dtype=np.float32)
    bo = np.asarray(bo, dtype=np.float32)

    tri = (np.arange(128)[:, None] <= np.arange(128)[None, :]).astype(np.float16)
    idm = np.eye(128, dtype=np.float16)
    zeros_bo = np.zeros_like(bo)
    in_maps = []
    for core in range(8):
        b = core // 2
        hg = core % 2
        cs = hg * 512
        m = attention_mask[b].astype(bool)
        keyb = np.where(m, 0.0, NEG).astype(np.float32).reshape(16, 128).T.copy()
        qmt = (m.astype(np.float32) / 32.0).reshape(16, 128).T.copy()
        xt = np.ascontiguousarray(x[b].T)
        xh = xt.astype(NP8)
        xl = ((xt - xh.astype(np.float32)) * 32.0).astype(NP8)

        def trio(ws):
            wh = ws.astype(NP8)
            wh32 = (wh.astype(np.float32) / 32.0).astype(NP8)
            wl32 = (ws - wh.astype(np.float32)).astype(NP8)
            return wh, wh32, wl32

        wqh, wqh32, wql32 = trio(np.ascontiguousarray(Wqkv[cs:cs + 512, :].T) * 32.0)
        wkh, wkh32, wkl32 = trio(
            np.ascontiguousarray(Wqkv[D + cs:D + cs + 512, :].T) * 32.0)
        wvh, wvh32, wvl32 = trio(
            np.ascontiguousarray(Wqkv[2 * D + cs:2 * D + cs + 512, :].T) * 32.0)
        in_maps.append({
            "xh8": xh,
            "xl8": xl,
            "wqh": wqh, "wqh32": wqh32, "wql32": wql32,
            "wkh": wkh, "wkh32": wkh32, "wkl32": wkl32,
            "wvh": wvh, "wvh32": wvh32, "wvl32": wvl32,
            "wo": np.ascontiguousarray(Wo[:, cs:cs + 512].T).astype(np.float16),
            "bqt": np.ascontiguousarray(bqkv[cs:cs + 512].reshape(NP, 128).T) * 32.0,
            "bkt": np.ascontiguousarray(
                bqkv[D + cs:D + cs + 512].reshape(NP, 128).T) * 32.0,
            "bv": bqkv[2 * D + cs:2 * D + cs + 512].copy() * 32.0,
            "bo": bo if hg == 0 else zeros_bo,
            "tri": tri,
            "keyb": keyb,
            "qmt": qmt,
            "idm": idm,
        })
    return in_maps


def kernel(x, attention_mask, Wqkv, bqkv, Wo, bo, _trace=False, _trace_kwargs=None):
    if "nc" not in _CACHE:
        _CACHE["nc"] = _build()
    nc = _CACHE["nc"]
    in_maps = _host_inputs(x, attention_mask, Wqkv, bqkv, Wo, bo)
    kwargs = {}
    if _trace:
        kwargs["trace"] = True
        if _trace_kwargs:
            kwargs.update(_trace_kwargs)
    res = run_bass_kernel_spmd(nc, in_maps, core_ids=list(range(8)), **kwargs)
    _CACHE["last_result"] = res
    out = np.empty((B, T, D), dtype=np.float32)
    for b in range(B):
        out[b] = res.results[2 * b]["out"] + res.results[2 * b + 1]["out"]
    return out


# revision 24
# speedup vs baseline: 1.2744x; 1.0472x over previous
"""Causal self-attention (B=4, T=2048, D=1024, H=16) on 8 Trainium2 NeuronCores.

Sharding: data-parallel over batch (4) x tensor-parallel over heads (2 groups
of 8 heads) = 8 cores. Each core computes q/k/v projections for its 8 heads,
head-local attention, and a partial out-projection; the host sums the two
partials per batch element (the out_proj all-reduce).

v2 design (vs the fp32r baseline):
  - fp16 matmul pipeline: x, Wq, Wk resident in SBUF as fp16; q/k projections,
    S = k^T q, P@V and the out-projection all run on fp16 operands
    (1 cycle/row with no fp32r >=256-column constraint, so causal diagonal
    tiles shrink to their exact width).
  - V projection via fp8e4 DoubleRow matmuls (2 contraction k-tiles per
    instruction at 0.5 cycles/row): 3-term residual split
    x@Wv ~= xh@wvh + xl@(wvh/32) + xh@(wvl) keeps the error ~2e-3 while
    cutting the V matmul time 4x. Wv is pre-scaled by 32 on the host
    (folded back via qmt = mask/32 in the epilogue).
  - Transposed P@V: out[128q, 65] per (head, q-tile, k-tile) with the P^T
    tile as stationary and V (plus a ones column for the softmax
    denominator) as moving operand: 65 rows/tile instead of ~512.
  - y normalized token-major with per-partition reciprocals, transposed
    back to dim-major via PE transpose (paired heads, [128,128] tiles),
    then one DVE copy per 512 tokens into fp16 yT for the out-projection.
  - qk projections of head-pair pr+1 and the out-projection are woven into
    the attention loop of pr to fill PE bubbles left by the exp activation
    (ACT is the phase-2 bottleneck at ~0.83 ns/row).
"""
import numpy as np
import ml_dtypes

import concourse.bass as bass
import concourse.bacc as bacc
import concourse.tile as tile
from concourse import mybir
from concourse.bass_utils import run_bass_kernel_spmd

F32 = mybir.dt.float32
F16 = mybir.dt.float16
F8 = mybir.dt.float8e4
DR = mybir.MatmulPerfMode.DoubleRow
EXP = mybir.ActivationFunctionType.Exp
MULT = mybir.AluOpType.mult
ADD = mybir.AluOpType.add

B, T, D = 4, 2048, 1024
H = 16            # total heads
HD = 64           # head dim
HC = 8            # heads per core
NP = 4            # head pairs per core
NEG = -1.0e6      # additive mask value (exp -> 0)
NP8 = np.dtype(ml_dtypes.float8_e4m3fn)

_CACHE = {}


def _build():
    nc = bacc.Bacc("TRN2", target_bir_lowering=False, debug=False,
                   enable_asserts=False)

    def din(name, shape, dt=F32):
        return nc.dram_tensor(name, shape, dt, kind="ExternalInput").ap()

    xh8 = din("xh8", [D, T], F8)
    xl8 = din("xl8", [D, T], F8)
    wqh = din("wqh", [D, 512], F8)
    wqh32 = din("wqh32", [D, 512], F8)
    wql32 = din("wql32", [D, 512], F8)
    wkh = din("wkh", [D, 512], F8)
    wkh32 = din("wkh32", [D, 512], F8)
    wkl32 = din("wkl32", [D, 512], F8)
    wvh = din("wvh", [D, 512], F8)
    wvh32 = din("wvh32", [D, 512], F8)
    wvl32 = din("wvl32", [D, 512], F8)
    wo = din("wo", [512, D], F16)
    bqt = din("bqt", [128, NP])
    bkt = din("bkt", [128, NP])
    bv = din("bv", [512])
    bo = din("bo", [D])
    constf = din("constf", [128, 40])       # keyb | qmt | bqt | bkt
    consth = din("consth", [128, 256], F16)  # tri | idm
    out = nc.dram_tensor("out", [T, D], F32, kind="ExternalOutput").ap()

    def bcast128(ap):
        return bass.AP(tensor=ap.tensor, offset=ap.offset, ap=[[0, 128]] + ap.ap)

    def dim0bc(ap, n):
        # insert a stride-0 dim after the partition dim (broadcast)
        return bass.AP(tensor=ap.tensor, offset=ap.offset,
                       ap=[ap.ap[0], [0, n]] + ap.ap[1:])

    with tile.TileContext(nc) as tc:
        cpool = tc.alloc_tile_pool(name="const", bufs=1)
        cf_t = cpool.tile([128, 40], F32)
        ch_t = cpool.tile([128, 256], F16)
        keyb_t = cf_t[:, 0:16]
        qmt_t = cf_t[:, 16:32]
        bqt_t = cf_t[:, 32:36]
        bkt_t = cf_t[:, 36:40]
        tri_t = ch_t[:, 0:128]
        idm_t = ch_t[:, 128:256]
        bvrep = cpool.tile([128, 512], F32)
        borep = cpool.tile([128, D], F32)
        vone_f = cpool.tile([128, HC, 16], F16)
        nc.vector.memset(vone_f, 1.0)

        wpool = tc.alloc_tile_pool(name="wts", bufs=1)
        wq_trio = [wpool.tile([128, 8, 512], F8, tag=f"wq{i}", name=f"wq{i}")
                   for i in range(3)]
        wk_trio = [wpool.tile([128, 8, 512], F8, tag=f"wk{i}", name=f"wk{i}")
                   for i in range(3)]
        wo_t = wpool.tile([128, 4, D], F16)

        xpool = tc.alloc_tile_pool(name="xp", bufs=1)
        xh_t = xpool.tile([128, 8, T], F8, tag="xh")
        xl_t = xpool.tile([128, 8, T], F8, tag="xl")

        qkvpool = tc.alloc_tile_pool(name="qkv", bufs=1)
        qT = qkvpool.tile([128, NP, T], F16, tag="qT")
        kT = qkvpool.tile([128, NP, T], F16, tag="kT")
        vA = qkvpool.tile([128, HC, 16, HD + 1], F16, tag="vA")
        nc.vector.tensor_copy(vA[:, :, :, HD], vone_f)

        ypool = tc.alloc_tile_pool(name="yT", bufs=1)
        yT = ypool.tile([128, NP, T], F16)

        def qk_chunks(pr, wt_trio, bias_t, outT, cch, pool):
            """One q/k projection psum group (3-term fp8 DR) + bias-add, split
            into 4 emission chunks (~430ns of PE work each)."""
            ps = [None]
            terms = ((wt_trio[0], xh_t), (wt_trio[1], xl_t), (wt_trio[2], xh_t))

            def mm_term(term):
                st, mv = terms[term]
                if term == 0:
                    ps[0] = pool.tile([128, 512], F32, tag=pool._qk_tag,
                                      name=f"qk{pr}_{outT is kT}_{cch}")
                for j in range(4):
                    nc.tensor.matmul(
                        ps[0], st[:, 2 * j:2 * j + 2, pr * 128:(pr + 1) * 128],
                        mv[:, 2 * j:2 * j + 2, cch * 512:(cch + 1) * 512],
                        start=(term == 0 and j == 0), stop=(term == 2 and j == 3),
                        perf_mode=DR)

            def bias():
                nc.vector.tensor_scalar_add(outT[:, pr, cch * 512:(cch + 1) * 512],
                                            ps[0], bias_t[:, pr:pr + 1])
            return [lambda t=t: mm_term(t) for t in range(3)] + [bias]

        # ---------------- prologue: V projection (fp8 DR) + q/k for pr=0 ----
        with (
            tc.tile_pool(name="wv8", bufs=1) as wv8pool,
            tc.tile_pool(name="psv", bufs=3, space="PSUM") as psv,
        ):
            psv._qk_tag = "pv"
            wvh_t = wv8pool.tile([128, 8, 512], F8, tag="wvh")
            wvh32_t = wv8pool.tile([128, 8, 512], F8, tag="wvh32")
            wvl32_t = wv8pool.tile([128, 8, 512], F8, tag="wvl32")

            # DMA plan, ordered by first use.
            #   sync:   wvh, xh chunks (+bvrep mid-way)
            #   scalar: wvh32, xl chunks, consts-f32
            #   gpsimd: wvl32, q/k weight trios, tri/idm, wo
            nc.sync.dma_start(out=wvh_t, in_=wvh.rearrange("(k p) n -> p k n", p=128))
            nc.scalar.dma_start(out=wvh32_t,
                                in_=wvh32.rearrange("(k p) n -> p k n", p=128))
            nc.gpsimd.dma_start(out=wvl32_t,
                                in_=wvl32.rearrange("(k p) n -> p k n", p=128))
            for ci in range(4):
                c0, c1 = ci * 512, (ci + 1) * 512
                nc.sync.dma_start(out=xh_t[:, :, c0:c1],
                                  in_=xh8[:, c0:c1].rearrange("(k p) t -> p k t", p=128))
                nc.scalar.dma_start(out=xl_t[:, :, c0:c1],
                                    in_=xl8[:, c0:c1].rearrange("(k p) t -> p k t", p=128))
                if ci == 0:
                    nc.sync.dma_start(out=bvrep, in_=bcast128(bv))
            for i, w in enumerate((wqh, wqh32, wql32)):
                nc.gpsimd.dma_start(out=wq_trio[i],
                                    in_=w.rearrange("(k p) n -> p k n", p=128))
            for i, w in enumerate((wkh, wkh32, wkl32)):
                nc.gpsimd.dma_start(out=wk_trio[i],
                                    in_=w.rearrange("(k p) n -> p k n", p=128))
            nc.scalar.dma_start(out=cf_t, in_=constf)
            nc.gpsimd.dma_start(out=ch_t, in_=consth)
            nc.scalar.dma_start(out=borep, in_=bcast128(bo))
            nc.gpsimd.dma_start(out=wo_t, in_=wo.rearrange("(g p) n -> p g n", p=128))

            # V projection: token-major, 3-term fp8 DoubleRow
            for tt in range(16):
                ps = psv.tile([128, 512], F32, tag="pv", name=f"pv{tt}")
                first = True
                for stat_t, mov_t in ((xh_t, wvh_t), (xl_t, wvh32_t), (xh_t, wvl32_t)):
                    for j in range(4):
                        nc.tensor.matmul(
                            ps, stat_t[:, 2 * j:2 * j + 2, tt * 128:(tt + 1) * 128],
                            mov_t[:, 2 * j:2 * j + 2, :],
                            start=first, stop=(stat_t is xh_t and mov_t is wvl32_t and j == 3),
                            perf_mode=DR)
                        first = False
                nc.vector.tensor_add(vA[:, :, tt, 0:HD],
                                     ps.rearrange("p (h d) -> p h d", h=HC),
                                     bvrep.rearrange("p (h d) -> p h d", h=HC))

            # q/k projections for pr=0, token chunk 0 only (the rest is woven
            # into the attention loop of pr=0)
            for wt_trio, bias_t, outT in ((wq_trio, bqt_t, qT), (wk_trio, bkt_t, kT)):
                for step in qk_chunks(0, wt_trio, bias_t, outT, 0, psv):
                    step()

        # ---------------- attention + interleaved qk(pr+1) / out-proj -------
        with (
            tc.tile_pool(name="ptile", bufs=8) as ppool,
            tc.tile_pool(name="ynp", bufs=2) as ynpool,
            tc.tile_pool(name="recp", bufs=2) as recpool,
            tc.tile_pool(name="obi", bufs=2) as obipool,
            tc.tile_pool(name="pss", bufs=2, space="PSUM") as pss,
            tc.tile_pool(name="psy", bufs=1, space="PSUM") as psy,
            tc.tile_pool(name="ptr", bufs=1, space="PSUM") as ptr,
            tc.tile_pool(name="paux", bufs=1, space="PSUM") as paux,
        ):
            paux._qk_tag = "aux"

            def qk_steps(pr, cch_list):
                """q/k projection step thunks for head-pair pr, given chunks."""
                for cch in cch_list:
                    for wt_trio, bias_t, outT in ((wq_trio, bqt_t, qT),
                                                  (wk_trio, bkt_t, kT)):
                        yield from qk_chunks(pr, wt_trio, bias_t, outT, cch, paux)

            done_oproj = []

            def oproj_steps(tt_list):
                """Out-projection step thunks for token tiles tt_list."""
                for tt in tt_list:
                    for oh in range(2):
                        po = [None]

                        def mk(gg, tt=tt, oh=oh, po=po):
                            def go():
                                if gg == 0:
                                    po[0] = paux.tile([128, 512], F32, tag="aux",
                                                      name=f"po{tt}_{oh}")
                                for g in (gg, gg + 1):
                                    nc.tensor.matmul(
                                        po[0], yT[:, g, tt * 128:(tt + 1) * 128],
                                        wo_t[:, g, oh * 512:(oh + 1) * 512],
                                        start=(g == 0), stop=(g == NP - 1))
                            return go
                        yield mk(0)
                        yield mk(2)

                        def fin(tt=tt, oh=oh, po=po):
                            ob = obipool.tile([128, 512], F32, tag="ob",
                                              name=f"ob{tt}_{oh}")
                            nc.vector.scalar_tensor_tensor(
                                ob, po[0], qmt_t[:, tt:tt + 1],
                                borep[:, oh * 512:(oh + 1) * 512], op0=MULT, op1=ADD)
                            eng = nc.sync if oh == 0 else nc.scalar
                            eng.dma_start(
                                out=out[tt * 128:(tt + 1) * 128,
                                        oh * 512:(oh + 1) * 512],
                                in_=ob)
                            if oh == 1:
                                done_oproj.append(tt)
                        yield fin

            class Filler:
                def __init__(self, queue):
                    self.queue = queue
                    self.consumed = 0

                def fill(self, n):
                    for _ in range(n):
                        step = next(self.queue, None)
                        if step is None:
                            return
                        step()
                        self.consumed += 1

                def force(self, upto):
                    self.fill(max(0, upto - self.consumed))

            import itertools
            prev_tail = None
            for pr in range(NP):
                if pr == 0:
                    # remaining qk chunks of pr=0, then all of pr=1
                    fl = Filler(itertools.chain(qk_steps(0, [1, 2, 3]),
                                                qk_steps(1, [0, 1, 2, 3])))
                    rate = 3
                elif pr < NP - 1:
                    fl = Filler(qk_steps(pr + 1, [0, 1, 2, 3]))
                    rate = 1
                else:
                    fl = Filler(iter(()))
                    rate = 3
                for qt in range(4):
                    q0 = qt * 512
                    nk = qt * 4 + 4
                    if pr == 0 and qt > 0:
                        # pr0 still needs its own q/k chunk qt before using it
                        fl.force(qt * 8)
                    if pr == NP - 1 and qt > 0:
                        # out-proj of token block qt-1 (yT complete for it)
                        tts = list(range((qt - 1) * 4, qt * 4))
                        fl.queue = itertools.chain(fl.queue, oproj_steps(tts))
                    ys = psy.tile([128, 2, 512], F32, tag="ys", name=f"ys{pr}_{qt}")
                    for kt in range(nk):
                        c = kt * 128 - q0
                        lo = max(c, 0)
                        smin = max(c // 128, 0)
                        s2 = pss.tile([128, 1024], F32, tag="s", name=f"s{pr}_{qt}_{kt}")
                        with tc.high_priority(offset=48):
                            for hh in range(2):
                                half = slice(hh * 64, hh * 64 + 64)
                                nc.tensor.matmul(s2[:, hh * 512 + lo:hh * 512 + 512],
                                                 kT[half, pr, kt * 128:(kt + 1) * 128],
                                                 qT[half, pr, q0 + lo:q0 + 512],
                                                 start=True, stop=True)
                        s2v = s2.rearrange("p (b n) -> p b n", b=2)
                        p2 = ppool.tile([128, 1024], F16, tag="p", name=f"p{pr}_{qt}_{kt}")
                        p2v = p2.rearrange("p (b n) -> p b n", b=2)
                        with tc.high_priority(offset=96):
                            nc.scalar.activation(p2v[:, :, lo:512],
                                                 s2v[:, :, lo:512], EXP,
                                                 bias=keyb_t[:, kt:kt + 1],
                                                 scale=0.125 / 1024.0)
                        if c >= 0:
                            nc.vector.tensor_mul(p2v[:, :, lo:lo + 128],
                                                 p2v[:, :, lo:lo + 128],
                                                 dim0bc(tri_t, 2))
                        if prev_tail is not None and kt == 1:
                            prev_tail()
                            prev_tail = None
                        for hh in range(2):
                            for s in range(smin, 4):
                                nc.tensor.matmul(
                                    ys[:, hh, s * 65:s * 65 + 65],
                                    p2[:, hh * 512 + s * 128:hh * 512 + (s + 1) * 128],
                                    vA[:, 2 * pr + hh, kt, :],
                                    start=(kt == 0 and s == 0),
                                    stop=(kt == qt * 4 + s),
                                    skip_group_check=True)
                        fl.fill(rate)

                    def tail(pr=pr, qt=qt, q0=q0, ys=ys):
                        denS = recpool.tile([128, 8], F32, tag="dn", name=f"dn{pr}_{qt}")
                        den_ap = bass.AP(tensor=ys.tensor, offset=ys.offset + 64,
                                         ap=[ys.ap[0], [512, 2], [65, 4]])
                        nc.vector.tensor_copy(denS, den_ap)
                        rec = recpool.tile([128, 8], F32, tag="rc", name=f"rc{pr}_{qt}")
                        nc.vector.reciprocal_approx_fast(rec, denS)
                        yn = ynpool.tile([128, 4, 128], F16, tag="yn", name=f"yn{pr}_{qt}")
                        tr = ptr.tile([128, 4, 128], F16, tag="tr", name=f"tr{pr}_{qt}")
                        for s in range(4):
                            out_ap = bass.AP(tensor=yn.tensor, offset=yn.offset + s * 128,
                                             ap=[yn.ap[0], [64, 2], [1, 64]])
                            in0 = bass.AP(tensor=ys.tensor, offset=ys.offset + s * 65,
                                          ap=[ys.ap[0], [512, 2], [1, 64]])
                            in1 = bass.AP(tensor=rec.tensor, offset=rec.offset + s,
                                          ap=[rec.ap[0], [4, 2], [0, 64]])
                            nc.vector.tensor_tensor(out_ap, in0, in1, op=MULT)
                        for s in range(4):
                            nc.tensor.matmul(tr[:, s, :], yn[:, s, :], idm_t,
                                             is_transpose=True)
                        nc.vector.tensor_copy(yT[:, pr, q0:q0 + 512],
                                              tr.rearrange("p s n -> p (s n)"))
                    if prev_tail is not None:
                        prev_tail()
                    prev_tail = tail
                # qk of the next pr must be complete before its attention
                if pr < NP - 1:
                    fl.fill(1000)
            prev_tail()

        # ---------------- out-projection tail --------------------------------
        with (
            tc.tile_pool(name="ob", bufs=4) as opool,
            tc.tile_pool(name="pso", bufs=4, space="PSUM") as psop,
        ):
            for tt in range(16):
                if tt in done_oproj:
                    continue
                for oh in range(2):
                    po = psop.tile([128, 512], F32, tag="o", name=f"pot{tt}_{oh}")
                    for g in range(NP):
                        nc.tensor.matmul(po, yT[:, g, tt * 128:(tt + 1) * 128],
                                         wo_t[:, g, oh * 512:(oh + 1) * 512],
                                         start=(g == 0), stop=(g == NP - 1))
                    ob = opool.tile([128, 512], F32, tag="obt", name=f"obt{tt}_{oh}")
                    nc.vector.scalar_tensor_tensor(ob, po, qmt_t[:, tt:tt + 1],
                                                   borep[:, oh * 512:(oh + 1) * 512],
                                                   op0=MULT, op1=ADD)
                    eng = nc.sync if oh == 0 else nc.scalar
                    eng.dma_start(out=out[tt * 128:(tt + 1) * 128,
                                          oh * 512:(oh + 1) * 512],
                                  in_=ob)
        for p in (ypool, qkvpool, xpool, wpool, cpool):
            p.release()
    nc.compile()
    return nc


def _f8(a):
    return np.asarray(a, dtype=np.float32).astype(NP8)


def _host_inputs(x, attention_mask, Wqkv, bqkv, Wo, bo):
    """Build the 8 per-core input maps."""
    x = np.asarray(x, dtype=np.float32)
    attention_mask = np.asarray(attention_mask)
    Wqkv = np.asarray(Wqkv, dtype=np.float32)
    bqkv = np.asarray(bqkv, dtype=np.float32)
    Wo = np.asarray(Wo, dtype=np.float32)
    bo = np.asarray(bo, dtype=np.float32)

    tri = (np.arange(128)[:, None] <= np.arange(128)[None, :]).astype(np.float16)
    idm = np.eye(128, dtype=np.float16)
    consth = np.concatenate([tri, idm], axis=1)
    zeros_bo = np.zeros_like(bo)
    in_maps = []
    for core in range(8):
        b = core // 2
        hg = core % 2
        cs = hg * 512
        m = attention_mask[b].astype(bool)
        keyb = np.where(m, 0.0, NEG).astype(np.float32).reshape(16, 128).T.copy()
        qmt = (m.astype(np.float32) / 32.0).reshape(16, 128).T.copy()
        xt = np.ascontiguousarray(x[b].T)
        xh = xt.astype(NP8)
        xl = ((xt - xh.astype(np.float32)) * 32.0).astype(NP8)

        def trio(ws):
            wh = ws.astype(NP8)
            wh32 = (wh.astype(np.float32) / 32.0).astype(NP8)
            wl32 = (ws - wh.astype(np.float32)).astype(NP8)
            return wh, wh32, wl32

        wqh, wqh32, wql32 = trio(np.ascontiguousarray(Wqkv[cs:cs + 512, :].T) * 32.0)
        wkh, wkh32, wkl32 = trio(
            np.ascontiguousarray(Wqkv[D + cs:D + cs + 512, :].T) * 32.0)
        wvh, wvh32, wvl32 = trio(
            np.ascontiguousarray(Wqkv[2 * D + cs:2 * D + cs + 512, :].T) * 32.0)
        in_maps.append({
            "xh8": xh,
            "xl8": xl,
            "wqh": wqh, "wqh32": wqh32, "wql32": wql32,
            "wkh": wkh, "wkh32": wkh32, "wkl32": wkl32,
            "wvh": wvh, "wvh32": wvh32, "wvl32": wvl32,
            "wo": np.ascontiguousarray(Wo[:, cs:cs + 512].T).astype(np.float16),
            "constf": np.concatenate(
                [keyb, qmt,
                 np.ascontiguousarray(bqkv[cs:cs + 512].reshape(NP, 128).T) * 32.0,
                 np.ascontiguousarray(
                     bqkv[D + cs:D + cs + 512].reshape(NP, 128).T) * 32.0],
                axis=1).astype(np.float32),
            "consth": consth,
            "bv": bqkv[2 * D + cs:2 * D + cs + 512].copy() * 32.0,
            "bo": bo if hg == 0 else zeros_bo,
        })
    return in_maps


def kernel(x, attention_mask, Wqkv, bqkv, Wo, bo, _trace=False, _trace_kwargs=None):
    if "nc" not in _CACHE:
        _CACHE["nc"] = _build()
    nc = _CACHE["nc"]
    in_maps = _host_inputs(x, attention_mask, Wqkv, bqkv, Wo, bo)
    kwargs = {}
    if _trace:
        kwargs["trace"] = True
        if _trace_kwargs:
            kwargs.update(_trace_kwargs)
    res = run_bass_kernel_spmd(nc, in_maps, core_ids=list(range(8)), **kwargs)
    _CACHE["last_result"] = res
    out = np.empty((B, T, D), dtype=np.float32)
    for b in range(B):
        out[b] = res.results[2 * b]["out"] + res.results[2 * b + 1]["out"]
    return out
